# revision 1
# baseline (speedup 1.0000x reference)
"""Llama4 MoE layer (top-1 routing) as an 8-core expert-parallel Trainium2 kernel.

Sharding: expert-parallel. Core e holds expert e's gate/up/down weights.
Tokens are dispatched to cores by the router's top-1 choice (the sharding
layer computes router logits + argmax and groups tokens per expert; this is
the "all-to-all token dispatch" of the expert-parallel strategy). Each core
runs the FFN for its ~T/E tokens, padded to a common capacity C so all cores
run one SPMD program. Outputs are scattered back to token order on the host.

Device math per core (C tokens, feature-on-partition layout):
    g[I,C] = gate_wᵀ·Xᵀ, u[I,C] = up_wᵀ·Xᵀ   (K = H, 16 k-tiles)
    a[I,C] = silu(g) * u                      (bf16)
    y[H,C] = down_wᵀ·a                        (K = I, 32 k-tiles)
Matmuls in bf16, PSUM accumulation in f32, output f32.
"""

import numpy as np
import ml_dtypes

import concourse.bass as bass
import concourse.mybir as mybir
import concourse.tile as tile
from concourse import bacc
from concourse.bass_utils import run_bass_kernel_spmd

SEQ, BS, H, I, E = 2048, 4, 2048, 4096, 8
N_CORES = 8
P = 128
NCHUNK = 512  # matmul moving free dim / PSUM bank width (f32)

BF16 = mybir.dt.bfloat16
F32 = mybir.dt.float32
np_bf16 = ml_dtypes.bfloat16

# Stash of the last BassKernelResults (exec_time_ns when BASS_TRACE=1).
last_results = None


def _build(C, h=H, i_dim=I):
    """Build + compile the per-core FFN program for capacity C tokens."""
    nkt = h // P      # gate/up k-tiles (= down m-tiles)
    nmt = i_dim // P  # gate/up m-tiles (= down k-tiles)
    chunks = []
    off = 0
    while off < C:
        sz = min(NCHUNK, C - off)
        chunks.append((off, sz))
        off += sz

    nc = bacc.Bacc("TRN2", target_bir_lowering=False, debug=False)
    xt_d = nc.dram_tensor("xt", [nkt, P, C], BF16, kind="ExternalInput")
    gw_d = nc.dram_tensor("gw", [nmt, P, nkt, P], BF16, kind="ExternalInput")
    uw_d = nc.dram_tensor("uw", [nmt, P, nkt, P], BF16, kind="ExternalInput")
    dw_d = nc.dram_tensor("dw", [nkt, P, nmt, P], BF16, kind="ExternalInput")
    out_d = nc.dram_tensor("out", [nkt, P, C], F32, kind="ExternalOutput")

    silu = mybir.ActivationFunctionType.Silu

    with tile.TileContext(nc) as tc:
        with (
            tc.tile_pool(name="xp", bufs=1) as xp,
            tc.tile_pool(name="wp", bufs=4) as wp,
            tc.tile_pool(name="ap", bufs=1) as ap,
            tc.tile_pool(name="sp", bufs=4) as sp,
            tc.tile_pool(name="op", bufs=4) as op,
            tc.tile_pool(name="pp", bufs=8, space="PSUM") as pp,
        ):
            # Warm the PE HAM clock gate with dummy matmuls while the first
            # DMAs are in flight (no data deps — zeros in, discarded out).
            warm = sp.tile([P, NCHUNK], BF16, tag="warm", bufs=1)
            nc.gpsimd.memset(warm[:], 0.0)
            warm_ps = pp.tile([P, NCHUNK], F32, tag="ps")
            for _ in range(16):
                nc.tensor.matmul(
                    warm_ps[:], warm[:, :P], warm[:], start=True, stop=True
                )

            xt_sb = xp.tile([P, nkt, C], BF16)
            # m=0/1 weights: split per k and interleave with the xt slices so
            # the first matmuls' inputs land within ~1µs each instead of
            # waiting for whole stripes behind the startup DMA flood.
            gw_t0 = wp.tile([P, nkt, P], BF16, tag="gw")
            uw_t0 = wp.tile([P, nkt, P], BF16, tag="uw")
            gw_t1 = wp.tile([P, nkt, P], BF16, tag="gw")
            uw_t1 = wp.tile([P, nkt, P], BF16, tag="uw")
            # gw0+xt are the only inputs of the first ~8µs of matmuls; uw0 is
            # first read ~8µs in and gw1 ~18µs in, so they follow.
            for k in range(nkt):
                nc.sync.dma_start(gw_t0[:, k, :], gw_d[0, :, k, :])
                nc.sync.dma_start(xt_sb[:, k, :], xt_d[k])
            for k in range(nkt):
                nc.sync.dma_start(uw_t0[:, k, :], uw_d[0, :, k, :])
            nc.sync.dma_start(gw_t1[:], gw_d[1])
            nc.sync.dma_start(uw_t1[:], uw_d[1])

            act_sb = ap.tile([P, nmt, C], BF16)

            # ---- gate/up + silu*up ----
            for m in range(nmt):
                if m == 0:
                    gw_t, uw_t = gw_t0, uw_t0
                elif m == 1:
                    gw_t, uw_t = gw_t1, uw_t1
                else:
                    gw_t = wp.tile([P, nkt, P], BF16, tag="gw")
                    nc.sync.dma_start(gw_t[:], gw_d[m])
                    uw_t = wp.tile([P, nkt, P], BF16, tag="uw")
                    nc.sync.dma_start(uw_t[:], uw_d[m])

                psg = [pp.tile([P, sz], F32, tag="ps", name=f"psg{m}_{ci}")
                       for ci, (_, sz) in enumerate(chunks)]
                for k in range(nkt):
                    for ci, (o, sz) in enumerate(chunks):
                        nc.tensor.matmul(
                            psg[ci][:],
                            gw_t[:, k, :],
                            xt_sb[:, k, o:o + sz],
                            start=(k == 0),
                            stop=(k == nkt - 1),
                        )
                psu = [pp.tile([P, sz], F32, tag="ps", name=f"psu{m}_{ci}")
                       for ci, (_, sz) in enumerate(chunks)]
                for k in range(nkt):
                    for ci, (o, sz) in enumerate(chunks):
                        nc.tensor.matmul(
                            psu[ci][:],
                            uw_t[:, k, :],
                            xt_sb[:, k, o:o + sz],
                            start=(k == 0),
                            stop=(k == nkt - 1),
                        )
                for ci, (o, sz) in enumerate(chunks):
                    sil = sp.tile([P, NCHUNK], F32, tag="sil")
                    nc.scalar.activation(sil[:, :sz], psg[ci][:], silu)
                    nc.vector.tensor_mul(
                        act_sb[:, m, o:o + sz], sil[:, :sz], psu[ci][:]
                    )

            # ---- down ----
            for m in range(nkt):
                dw_t = wp.tile([P, nmt, P], BF16, tag="dw")
                nc.sync.dma_start(dw_t[:], dw_d[m])
                psd = [pp.tile([P, sz], F32, tag="ps", name=f"psd{m}_{ci}")
                       for ci, (_, sz) in enumerate(chunks)]
                if m < nkt - 1:
                    for k in range(nmt):
                        for ci, (o, sz) in enumerate(chunks):
                            nc.tensor.matmul(
                                psd[ci][:],
                                dw_t[:, k, :],
                                act_sb[:, k, o:o + sz],
                                start=(k == 0),
                                stop=(k == nmt - 1),
                            )
                    for ci, (o, sz) in enumerate(chunks):
                        ot = op.tile([P, NCHUNK], F32, tag="ot")
                        nc.vector.tensor_copy(ot[:, :sz], psd[ci][:])
                        nc.sync.dma_start(out_d[m][:, o:o + sz], ot[:, :sz])
                else:
                    # last m-tile: finish chunks one at a time so their output
                    # DMAs drain before the exit barrier, not after the very
                    # last matmul (trades ~0.5µs of same-bank streaming for
                    # ~2µs of tail).
                    for ci, (o, sz) in enumerate(chunks):
                        for k in range(nmt):
                            nc.tensor.matmul(
                                psd[ci][:],
                                dw_t[:, k, :],
                                act_sb[:, k, o:o + sz],
                                start=(k == 0),
                                stop=(k == nmt - 1),
                            )
                        ot = op.tile([P, NCHUNK], F32, tag="ot")
                        nc.vector.tensor_copy(ot[:, :sz], psd[ci][:])
                        nc.sync.dma_start(out_d[m][:, o:o + sz], ot[:, :sz])

    nc.compile()
    return nc


def kernel(hidden_states, router_w, gate_w, up_w, down_w):
    global last_results
    X = np.asarray(hidden_states, dtype=np.float32).reshape(-1, H)
    router_w = np.asarray(router_w, dtype=np.float32)
    gate_w = np.asarray(gate_w, dtype=np.float32)
    up_w = np.asarray(up_w, dtype=np.float32)
    down_w = np.asarray(down_w, dtype=np.float32)
    T = X.shape[0]

    # --- token dispatch (sharding layer): top-1 expert per token ---
    # Mirror the reference's routing computation op-for-op (jnp.einsum +
    # argmax) so near-tied logits resolve to the same expert.
    import jax.numpy as jnp

    logits = jnp.einsum(
        "sbh,he->sbe", np.asarray(hidden_states, dtype=np.float32), router_w
    )
    eid = np.asarray(jnp.argmax(logits, axis=-1)).reshape(-1)  # [T]
    idx = [np.nonzero(eid == e)[0] for e in range(E)]
    max_count = max(len(ix) for ix in idx)
    # C is only ever a free-axis width (tiles/DMA/PSUM), so it needs no
    # 128-alignment; a multiple of 4 keeps bf16/f32 rows 8-byte aligned.
    C = max(64, ((max_count + 3) // 4) * 4)

    nkt, nmt = H // P, I // P
    in_maps = []
    for e in range(E):
        ix = idx[e]
        Xe = np.zeros((C, H), np.float32)
        Xe[: len(ix)] = X[ix]
        xt = np.ascontiguousarray(Xe.T).reshape(nkt, P, C).astype(np_bf16)
        gw = gate_w[e].reshape(nkt, P, nmt, P).transpose(2, 1, 0, 3).astype(np_bf16)
        uw = up_w[e].reshape(nkt, P, nmt, P).transpose(2, 1, 0, 3).astype(np_bf16)
        dw = down_w[e].reshape(nmt, P, nkt, P).transpose(2, 1, 0, 3).astype(np_bf16)
        in_maps.append({"xt": xt, "gw": gw, "uw": uw, "dw": dw})

    nc = _build(C)
    last_results = run_bass_kernel_spmd(nc, in_maps, list(range(N_CORES)))

    out = np.zeros((T, H), np.float32)
    for e in range(E):
        ix = idx[e]
        oe = last_results.results[e]["out"].reshape(H, C)
        out[ix] = oe[:, : len(ix)].T
    return out.reshape(SEQ, BS, H)

